# revision 56
# baseline (speedup 1.0000x reference)
"""Distributed single-head attention block for trn2 (8 NeuronCores), v16.

reference:
    q = x @ Wq.T + bq ; k = x @ Wk.T + bk ; v = x @ Wv.T + bv
    out = x + softmax(q @ k.T / sqrt(D)) @ v       x: [4, 2048, 1024]

Sharding: 8 cores = 4 batches x 2 halves. Core c owns batch c//2 and
rows [h*1024, (h+1)*1024) with h = c%2 — both as queries and as keys.
Each core projects Q and V for its OWN half and computes scores via
the reassociation scores = Q.Kt = (Q.Wk).Xt: Q.Wk is local (same MACs
as a half-K projection) and the full Xt is a plain host input — the K
exchange disappears entirely. Only V is exchanged (one pairwise 2MB
AllGather, natural batch order), triggered at ~80us with ~20us of
margin before attention needs it.

Scheduling model (measured): the tile scheduler places every
instruction as early in its engine's in-order queue as the EMISSION
positions of its dependencies allow — emission order is only a
tiebreak. A dma_start costs ~0.7us of issue time on the triggering
engine's queue, and a blocked DMA at the gpsimd queue head stalls the
collective handshake machinery (the CC stream runs on the gpsimd
cores), so:
    sync   : input loads, then the 4 big slot readbacks (K before V)
    scalar : bq load + K/V stage-outs (early), exp activations, output
             stores
    vector : psum->SBUF casts and all other DVE math only
    gpsimd : ONLY the two collective triggers
Consecutive collectives serialize on the single CC stream including
~15us of per-op trigger latency: K (needed at ~93us) goes first, V
(needed at ~160us) second. The qT loop is ec-outer so the first
hoisted score chain keeps 15/16 of the qT chains as PE cover while the
K AllGather lands. Scores and attention share ONE psum ring (same
tag), which both gives scores 4-deep exp backpressure slack and pins
hoisted attention chains to ~pass-1 end, after the V readback.

Device-side layouts (host pre-transposes + bf16-casts so the
contraction dim always lands on SBUF partitions):
    xqT  [D, SQ]            bf16  x[b, half].T   -> Q/K/V projections
    wvT  [D, D]             bf16  Wv.T
    wkE  [EC, 128, DC, 128] bf16  Wk.T e-chunk-major (wkE[ec,p,dc,j]
          = Wk.T[dc*128+p, ec*128+j]) so the first kT chain only needs
          xqT + one 256KB e-chunk: first real matmul at ~8us.
    wqE  same for Wq.T
The device returns softmax(qk/sqrt(D))@v per owned half; the host adds
the residual x + bv (attention weights sum to 1, so the V bias is a
plain output offset; the K bias cancels in softmax; only the Q bias is
applied on-device). Projections emit qT/kT [e, s] (scores contraction
over e) and v [s, e] (attn contraction over keys). Softmax rows live
on partitions: exp on ScalarE with accum_out giving row sums for free;
no max subtraction (scores are O(10) for this model so exp cannot
overflow in f32). P is transposed 128x128 on TensorE (identity
matmul); transposes are emitted TWO score-chains late so the PE never
waits on the exp that produces their input. Score order is slot-0 key
chunks for all q-tiles first (slot-0 readback lands first), then kc in
{2,3} per q-tile so each tile's softmax reciprocal is ready before its
attention epilogue.

Measured: 225.3us vs the 254.0us K-replicated baseline.
Budget: ~16.5us startup (boot + clock-ramp warmup + first input DMAs,
xqT spread over the three DMA-capable queues), ~85us projections, ~4us
waiting the K AllGather readbacks, ~120us saturated
scores+transposes+attention, ~4us drain. Rejected variants, measured
slower: masked-ReduceScatter partner exchange (DVE mask ops became the
bottleneck), transposed-scores with ones-matmul softmax denominators
(interleaved [1,512] accumulation chains serialize the PE), mm_ps=5 +
tr_ps=3 psum split, xqT piece 3 on the scalar queue (delays the
K-stage path). Note: the part runs bimodally — ~228-231us at 2.37GHz,
~271us when HAM power-throttled to ~2.0GHz; idle time restores it.
"""

import numpy as np

B, S, D = 4, 2048, 1024
SQ = S // 2  # queries/keys owned per core
NCORES = 8
DC = D // 128  # contraction chunks
EC = D // 128  # embed chunks
SC = S // 128  # key chunks, full batch
SCH = SQ // 128  # key chunks per half
QT = SQ // 128  # query tiles per core
KC = S // 512  # score column chunks, full batch
KCH = SQ // 512  # score column chunks per half
EJ = D // 512  # 512-wide embed column chunks

_cache = {}


def _build():
    import concourse.bass as bass
    import concourse.tile as tile
    from concourse import bacc, mybir
    from concourse.masks import make_identity

    f32 = mybir.dt.float32
    bf16 = mybir.dt.bfloat16
    Alu = mybir.AluOpType
    Act = mybir.ActivationFunctionType

    nc = bacc.Bacc(None, target_bir_lowering=False, debug=False)

    xqT_d = nc.declare_dram_parameter("xqT", [D, SQ], bf16, isOutput=False)
    xT_d = nc.declare_dram_parameter("xT", [D, S], bf16, isOutput=False)
    wqE_d = nc.declare_dram_parameter("wqE", [EC, 128, DC, 128], bf16, isOutput=False)
    wkQ_d = nc.declare_dram_parameter("wkQ", [EC, 128, DC, 128], bf16, isOutput=False)
    wv_d = nc.declare_dram_parameter("wvT", [D, D], bf16, isOutput=False)
    bq_d = nc.declare_dram_parameter("bq", [D], f32, isOutput=False)
    out_d = nc.declare_dram_parameter("out", [SQ, D], f32, isOutput=True)

    # Pairwise V exchange staging; AllGather slot order = natural batch
    # order, identical on both pair members (uniform SPMD graph). K needs
    # NO exchange: scores = Q.Kt = (Q.Wk).Xt, so each core computes Q.Wk
    # locally (same MACs as its half-K projection) and contracts against
    # the full Xt, which is a plain host input.
    vx_in = nc.dram_tensor("vx_in", [SCH, 128, D], bf16)
    vx_out = nc.dram_tensor("vx_out", [2, SCH, 128, D], bf16)

    groups = [[0, 1], [2, 3], [4, 5], [6, 7]]

    with tile.TileContext(nc) as tc:
        with tc.tile_pool(name="pers", bufs=1) as pers:
            qT_sb = pers.tile([128, EC, SQ], bf16, tag="qT")
            xT_sb = pers.tile([128, DC, S], bf16, tag="xT")
            qwkT_sb = pers.tile([128, DC, SQ], bf16, tag="qwkT")
            v_sb = pers.tile([128, 2, SCH, D], bf16, tag="v")
            ident = pers.tile([128, 128], bf16, tag="ident")
            make_identity(nc, ident)
            bq_sb = pers.tile([128, EC], f32, tag="bq")

            # PE warmup: dense dummy matmuls while the first input DMAs land,
            # so the HAM clock gate is already ramped when real work starts.
            warm_sb = pers.tile([128, 512], bf16, tag="warm")
            warm_dump = pers.tile([128, 512], f32, tag="warm_dump")
            nc.vector.memset(warm_sb, 0.0)
            with tc.tile_pool(name="warm_ps", bufs=1, space="PSUM") as warm_ps:
                wps = warm_ps.tile([128, 512], f32, tag="wps")
                NWARM = 10
                for i in range(NWARM):
                    nc.tensor.matmul(
                        wps,
                        lhsT=warm_sb[:, 0:128],
                        rhs=warm_sb,
                        start=(i == 0),
                        stop=(i == NWARM - 1),
                    )
                nc.vector.tensor_copy(out=warm_dump, in_=wps)

            bq_ap = bq_d.ap()
            nc.scalar.dma_start(
                out=bq_sb,
                in_=bass.AP(tensor=bq_ap.tensor, offset=0, ap=[[1, 128], [128, EC]]),
            )

            with (
                tc.tile_pool(name="ld", bufs=1) as ld,
                tc.tile_pool(name="stage", bufs=4) as stage,
                tc.tile_pool(name="proj_ps", bufs=6, space="PSUM") as proj_ps,
            ):
                xqT_sb = ld.tile([128, DC, SQ], bf16, tag="xqT")
                wk_sb = ld.tile([128, EC, DC, 128], bf16, tag="wk")
                wq_sb = ld.tile([128, EC, DC, 128], bf16, tag="wq")
                wv_sb = ld.tile([128, DC, D], bf16, tag="wv")
                xT_ap = xT_d.ap()

                # DMA priority: xqT gates every projection — split across
                # TWO queues (sync + scalar) since one queue sustains only
                # ~200GB/s; wk per-ec chunks so the first kT chain starts
                # after xqT + 256KB; wv/wq as single big strided DMAs. A
                # tiny leading DMA warms the cold sync ring so the first
                # big transfer runs at full rate.
                warmdma = ld.tile([128, 1], f32, tag="warmdma")
                nc.sync.dma_start(
                    out=warmdma,
                    in_=bass.AP(
                        tensor=bq_d.ap().tensor, offset=0, ap=[[1, 128], [128, 1]]
                    ),
                )
                xqT_ap = xqT_d.ap()
                # xqT in 4 pieces across ALL FOUR queues (one queue moves
                # only ~200GB/s): a dep-free DMA at the gpsimd head never
                # waits so it cannot stall the collective machinery, and the
                # vector-queue trigger follows the warm memset. wk0 leads on
                # sync — the first kT chain needs it plus whichever xqT
                # pieces its matmuls touch (per-region deps).
                def xqT_piece(q, ci):
                    q.dma_start(
                        out=xqT_sb[:, ci * 2 : ci * 2 + 2, :],
                        in_=bass.AP(
                            tensor=xqT_ap.tensor,
                            offset=ci * 2 * 128 * SQ,
                            ap=[[SQ, 128], [128 * SQ, 2], [1, SQ]],
                        ),
                    )

                nc.sync.dma_start(out=wq_sb[:, 0, :, :], in_=wqE_d[0])
                xqT_piece(nc.sync, 0)
                xqT_piece(nc.scalar, 1)
                xqT_piece(nc.scalar, 2)
                xqT_piece(nc.gpsimd, 3)
                for ec in range(1, EC):
                    nc.sync.dma_start(out=wq_sb[:, ec, :, :], in_=wqE_d[ec])
                nc.sync.dma_start(
                    out=wv_sb,
                    in_=bass.AP(
                        tensor=wv_d.ap().tensor,
                        offset=0,
                        ap=[[D, 128], [128 * D, DC], [1, D]],
                    ),
                )
                nc.sync.dma_start(
                    out=wk_sb,
                    in_=bass.AP(
                        tensor=wkQ_d.ap().tensor,
                        offset=0,
                        ap=[[DC * 128, 128], [128 * DC * 128, EC], [1, DC * 128]],
                    ),
                )
                # full Xt for the score contraction: needed only when pass 1
                # starts (~105us), so it rides at the END of the in-order
                # sync queue — after the startup-critical loads, never
                # stealing bandwidth from them.
                for half in range(2):
                    nc.sync.dma_start(
                        out=xT_sb[:, half * (DC // 2) : (half + 1) * (DC // 2), :],
                        in_=bass.AP(
                            tensor=xT_ap.tensor,
                            offset=half * (DC // 2) * 128 * S,
                            ap=[[S, 128], [128 * S, DC // 2], [1, S]],
                        ),
                    )

                # qT[e, sq] = sum_d wqT[d, e] * xqT[d, sq]  (+bq per-partition)
                # FIRST projection: it gates Q.Wk which gates scores.
                for ec in range(EC):
                    for j in range(SQ // 512):
                        jsl = slice(j * 512, (j + 1) * 512)
                        ps = proj_ps.tile([128, 512], f32, tag="ps")
                        for dc in range(DC):
                            nc.tensor.matmul(
                                ps,
                                lhsT=wq_sb[:, ec, dc, :],
                                rhs=xqT_sb[:, dc, jsl],
                                start=(dc == 0),
                                stop=(dc == DC - 1),
                            )
                        nc.vector.tensor_scalar_add(
                            out=qT_sb[:, ec, jsl],
                            in0=ps,
                            scalar1=bq_sb[:, ec : ec + 1],
                        )

                # v own half [sk_own, e], same pattern.
                for sc in range(SCH):
                    vst = stage.tile([128, D], bf16, tag="vst")
                    for j in range(EJ):
                        jsl = slice(j * 512, (j + 1) * 512)
                        ps = proj_ps.tile([128, 512], f32, tag="ps")
                        for dc in range(DC):
                            nc.tensor.matmul(
                                ps,
                                lhsT=xqT_sb[:, dc, sc * 128 : (sc + 1) * 128],
                                rhs=wv_sb[:, dc, jsl],
                                start=(dc == 0),
                                stop=(dc == DC - 1),
                            )
                        nc.vector.tensor_copy(out=vst[:, jsl], in_=ps)
                    nc.scalar.dma_start(out=vx_in[sc], in_=vst)
                nc.gpsimd.collective_compute(
                    "AllGather",
                    Alu.bypass,
                    replica_groups=groups,
                    ins=[vx_in.ap().opt()],
                    outs=[vx_out.ap().opt()],
                )
                # attn chain (qt, j2) reads both slots of column block j2:
                # read back j-outer, slot-inner so block 0 completes first.
                vx_out_ap = vx_out.ap()
                for j in range(EJ):
                    for s in range(2):
                        nc.sync.dma_start(
                            out=v_sb[:, s, :, j * 512 : (j + 1) * 512],
                            in_=bass.AP(
                                tensor=vx_out_ap.tensor,
                                offset=s * (SCH * 128 * D) + j * 512,
                                ap=[[D, 128], [128 * D, SCH], [1, 512]],
                            ),
                        )

                # QWkT[d, q] = sum_e Wk[e, d] * qT[e, q] — the locally
                # computable half of scores = (Q.Wk).Xt. Same MACs as the
                # half-K projection it replaces; the proj_ps ring (shared
                # with the v chains above) keeps these behind V staging so
                # the V exchange still triggers early.
                for dc in range(DC):
                    for j in range(SQ // 512):
                        jsl = slice(j * 512, (j + 1) * 512)
                        ps = proj_ps.tile([128, 512], f32, tag="ps")
                        for ec in range(EC):
                            nc.tensor.matmul(
                                ps,
                                lhsT=wk_sb[:, ec, dc, :],
                                rhs=qT_sb[:, ec, jsl],
                                start=(ec == 0),
                                stop=(ec == EC - 1),
                            )
                        nc.vector.tensor_copy(out=qwkT_sb[:, dc, jsl], in_=ps)

            with (
                tc.tile_pool(name="att", bufs=2) as att,
                tc.tile_pool(name="small", bufs=2) as small,
                tc.tile_pool(name="mm_ps", bufs=4, space="PSUM") as mm_ps,
                tc.tile_pool(name="tr_ps", bufs=4, space="PSUM") as tr_ps,
            ):
                inv_sqrt_d = float(1.0 / np.sqrt(D))
                P_list = [
                    att.tile([128, S], bf16, name=f"P{qt}", tag=f"P{qt}", bufs=1)
                    for qt in range(QT)
                ]
                PT_list = [
                    att.tile(
                        [128, SC, 128], bf16, name=f"PT{qt}", tag=f"PT{qt}", bufs=1
                    )
                    for qt in range(QT)
                ]
                den4_list = [
                    small.tile([128, KC], f32, name=f"den4{qt}", tag=f"den4{qt}", bufs=1)
                    for qt in range(QT)
                ]
                recip_list = [
                    small.tile(
                        [128, 1], f32, name=f"recip{qt}", tag=f"recip{qt}", bufs=1
                    )
                    for qt in range(QT)
                ]

                # qt-outer: everything is local now (no collective on the
                # score path), so each q-tile's den/recip completes right
                # after its 4 chunks — well before its attention epilogue.
                units = [(kc, qt) for qt in range(QT) for kc in range(KC)]

                def emit_transposes(kc, qt):
                    for j in range(kc * 4, kc * 4 + 4):
                        tp = tr_ps.tile([128, 128], bf16, tag="tr")
                        nc.tensor.transpose(
                            tp, P_list[qt][:, j * 128 : (j + 1) * 128], ident
                        )
                        nc.vector.tensor_copy(out=PT_list[qt][:, j, :], in_=tp)

                for i, (kc, qt) in enumerate(units):
                    csl = slice(kc * 512, (kc + 1) * 512)
                    qsl = slice(qt * 128, (qt + 1) * 128)
                    ps = mm_ps.tile([128, 512], f32, tag="mm")
                    for dc in range(DC):
                        nc.tensor.matmul(
                            ps,
                            lhsT=qwkT_sb[:, dc, qsl],
                            rhs=xT_sb[:, dc, csl],
                            start=(dc == 0),
                            stop=(dc == DC - 1),
                        )
                    nc.scalar.activation(
                        out=P_list[qt][:, csl],
                        in_=ps,
                        func=Act.Exp,
                        scale=inv_sqrt_d,
                        accum_out=den4_list[qt][:, kc : kc + 1],
                    )
                    if i >= 2:
                        emit_transposes(*units[i - 2])
                    if kc == KC - 1:
                        den = small.tile([128, 1], f32, tag="den", bufs=4)
                        nc.vector.reduce_sum(
                            out=den, in_=den4_list[qt], axis=mybir.AxisListType.X
                        )
                        nc.vector.reciprocal(recip_list[qt], den)
                emit_transposes(*units[-2])
                emit_transposes(*units[-1])

                # pass 2: attn + scaled epilogue per q-tile. The psum tiles
                # come from the SAME ring as the score psums, pinning these
                # chains behind pass 1 in the PE stream (after the V
                # readback). Residual is added on the host.
                for qt in range(QT):
                    qsl = slice(qt * 128, (qt + 1) * 128)
                    PT_sb = PT_list[qt]
                    recip = recip_list[qt]
                    ot = att.tile([128, D], f32, tag="ot", bufs=3)
                    for j2 in range(EJ):
                        jsl = slice(j2 * 512, (j2 + 1) * 512)
                        pa = mm_ps.tile([128, 512], f32, tag="mm")
                        for j in range(SC):
                            nc.tensor.matmul(
                                pa,
                                lhsT=PT_sb[:, j, :],
                                rhs=v_sb[:, j // SCH, j % SCH, jsl],
                                start=(j == 0),
                                stop=(j == SC - 1),
                            )
                        nc.vector.tensor_scalar_mul(
                            out=ot[:, jsl], in0=pa, scalar1=recip
                        )
                        nc.scalar.dma_start(out=out_d[qsl, jsl], in_=ot[:, jsl])

    nc.compile()
    return nc


def _get_nc():
    if "nc" not in _cache:
        _cache["nc"] = _build()
    return _cache["nc"]


def kernel(embedded, Wq, bq, Wk, bk, Wv, bv):
    import ml_dtypes

    from concourse.bass_utils import run_bass_kernel_spmd

    bf16 = ml_dtypes.bfloat16
    x = np.ascontiguousarray(np.asarray(embedded, dtype=np.float32))
    Wq = np.asarray(Wq, dtype=np.float32)
    Wk = np.asarray(Wk, dtype=np.float32)
    Wv = np.asarray(Wv, dtype=np.float32)
    bq = np.ascontiguousarray(np.asarray(bq, dtype=np.float32))
    bk = np.ascontiguousarray(np.asarray(bk, dtype=np.float32))
    bv = np.ascontiguousarray(np.asarray(bv, dtype=np.float32))

    # e-chunk-major weight layouts: wE[ec, p, dc, j] = W.T[dc*128+p, ec*128+j]
    def echunk(wT):
        return np.ascontiguousarray(
            wT.reshape(DC, 128, EC, 128).transpose(2, 1, 0, 3)
        )

    wqT = np.ascontiguousarray(Wq.T).astype(bf16)
    wvT = np.ascontiguousarray(Wv.T).astype(bf16)
    wqE = echunk(wqT)
    # Wk NATURAL orientation, e-chunk-major: wkQ[ec,p,dc,j] =
    # Wk[ec*128+p, dc*128+j] (contraction over e for Q.Wk) — a plain
    # reshape, no transpose.
    wkQ = np.ascontiguousarray(Wk.astype(bf16).reshape(EC, 128, DC, 128))
    xT = [np.ascontiguousarray(x[b].T).astype(bf16) for b in range(B)]

    in_maps = []
    for c in range(NCORES):
        b, h = c // 2, c % 2
        qs = slice(h * SQ, (h + 1) * SQ)
        in_maps.append(
            {
                "xqT": np.ascontiguousarray(xT[b][:, qs]),
                "xT": xT[b],
                "wqE": wqE,
                "wkQ": wkQ,
                "wvT": wvT,
                "bq": bq,
            }
        )

    _cache["in_maps"] = in_maps
    nc = _get_nc()
    res = run_bass_kernel_spmd(nc, in_maps, core_ids=list(range(NCORES)))
    out = np.empty((B, S, D), dtype=np.float32)
    for c in range(NCORES):
        b, h = c // 2, c % 2
        out[b, h * SQ : (h + 1) * SQ, :] = res.results[c]["out"]
    # residual (+ V bias, which passes through the attention average)
    out += x + bv
    return out
